# revision 13
# baseline (speedup 1.0000x reference)
"""Trainium2 Bass kernel for nn_EntropyMaskGate (fp32r 3-term split version).

Per core (2 images, batch-sharded over 8 cores):
  conv1 (1x1, 256->64) -> gelu -> grouped 3x3 conv (SAME, 8 groups) -> gelu
  -> conv3 (1x1, 64->256) + bias = entropy_scores -> 2x2 block sums ->
  per-(b,c)-row 256th-smallest threshold -> u8 block mask (host upsamples).

All matmuls run in fp32r (PE rounds operands to 11 explicit mantissa bits,
1 cyc/row vs fp32's 4).  Exactness is recovered with a 3-term split:
    w @ x = wh@xh + wh@xl + wl@xh + O(2^-23)
where wh = rnd11(w), wl = rnd11(w - wh) (host-precomputed, DMA'd straight
into fp32r dram tensors) and xh = rnd11(x), xl = rnd11(x - xh) (features
split on host; h1/h2 split on device: ACT writes gelu to fp32r = xh, DVE
subtracts to an fp32r tile = xl).  Stacking [xh | xl] in the 128-partition
K dim doubles K utilisation for the grouped conv2 (K=64 blocks) and conv3
(K=64), so the 3x cost of the split is mostly absorbed.

Threshold search: per-image 2-chain bisection on normalized bf16 copies;
chain 0 runs self-contained on ACT (Sign count w/ accum + Sign + Identity
mid update), chain 1 self-contained on DVE, so neither blocks the other
engine's queue.  Search work is diced into batches and dripped into later
conv phases: image 0's search hides under image 1's convs, and image 1's
search is handed to the NEXT repeat's image-0 conv phase (steady-state
software pipelining across repeats; the final repeat's tail is flushed at
the end).  An exact f32 top-8 finisher recovers the true threshold.
"""

import numpy as np

import concourse.bass as bass
import concourse.mybir as mybir
from concourse import bacc, bass_utils
from concourse.tile import TileContext

F32 = mybir.dt.float32
F32R = mybir.dt.float32r
BF16 = mybir.dt.bfloat16
I32 = mybir.dt.int32
U8 = mybir.dt.uint8
AF = mybir.ActivationFunctionType
OP = mybir.AluOpType

B, C, H, W = 16, 256, 64, 64
MID, GROUPS = 64, 8
N_CORES = 8
IMGS = B // N_CORES
HW = H * W                   # 4096
NBLK = 1024
KEEP = 256
T_BISECT = 9
BR_LO = -1.05
BR_HI = -0.35
K_SEARCH = 253
S_TARGET = float(NBLK - 2 * K_SEARCH)   # sign-sum equivalent of cnt>=K_SEARCH
BIGNEG = -(2.0 ** 96)
HALO = 66

TRACE = False
LAST_RESULTS = None
import os as _os
DBG_NOSEARCH = int(_os.environ.get("KM_NOSEARCH", "0"))
DBG_TBI = int(_os.environ.get("KM_TBI", "-1"))
DBG_NOFIN = int(_os.environ.get("KM_NOFIN", "0"))


def _round11(a):
    """Round-to-nearest (half-up) keeping 11 explicit mantissa bits."""
    b = np.ascontiguousarray(a, np.float32).view(np.uint32).astype(np.uint64)
    b = ((b + (1 << 11)) >> 12) << 12
    return (b & 0xFFFFFFFF).astype(np.uint32).view(np.float32)


def _split11(a):
    hi = _round11(a)
    lo = _round11(np.asarray(a, np.float32) - hi)
    return hi, lo


def build_nc(repeat=1):
    nc = bacc.Bacc("TRN2", target_bir_lowering=False, debug=False,
                   num_devices=N_CORES)

    d = {}
    d["fh"] = nc.dram_tensor("fh", [IMGS, C, HW], F32R, kind="ExternalInput").ap()
    d["fl"] = nc.dram_tensor("fl", [IMGS, C, HW], BF16, kind="ExternalInput").ap()
    d["wh1"] = nc.dram_tensor("wh1", [128, 2, MID], F32R, kind="ExternalInput").ap()
    d["wl1"] = nc.dram_tensor("wl1", [128, 2, MID], F32R, kind="ExternalInput").ap()
    d["t1w"] = nc.dram_tensor("t1w", [128, 9, MID], F32R, kind="ExternalInput").ap()
    d["t2w"] = nc.dram_tensor("t2w", [128, 3, MID], F32R, kind="ExternalInput").ap()
    d["w2s"] = nc.dram_tensor("w2s", [MID, 3, MID], F32R, kind="ExternalInput").ap()
    d["w3p"] = nc.dram_tensor("w3p", [128, C], F32R, kind="ExternalInput").ap()
    d["wl3"] = nc.dram_tensor("wl3", [MID, C], F32R, kind="ExternalInput").ap()
    d["b1"] = nc.dram_tensor("b1", [MID, 1], F32, kind="ExternalInput").ap()
    d["b2"] = nc.dram_tensor("b2", [MID, 1], F32, kind="ExternalInput").ap()
    d["b3s"] = nc.dram_tensor("b3s", [128, 2], F32, kind="ExternalInput").ap()
    d["scores"] = nc.dram_tensor("scores", [IMGS, C, HW], BF16, kind="ExternalOutput").ap()
    d["mask"] = nc.dram_tensor("mask", [IMGS, C, NBLK], U8, kind="ExternalOutput").ap()

    with TileContext(nc) as tc:
        ctx = _make_ctx(nc, tc, d)
        pending = []
        for _rep in range(repeat):
            pending = _build(nc, ctx, pending)
        for b in pending:
            b()
        for p in ctx["pools"]:
            p.release()
    nc.compile()
    return nc


def _make_ctx(nc, tc, d):
    cpool = tc.alloc_tile_pool(name="consts", bufs=1)
    xpool = tc.alloc_tile_pool(name="x", bufs=2)
    h1pool = tc.alloc_tile_pool(name="h1", bufs=1)
    h2pool = tc.alloc_tile_pool(name="h2", bufs=1)
    sfpool = tc.alloc_tile_pool(name="sf", bufs=2)
    ipool = tc.alloc_tile_pool(name="impp", bufs=1)
    bpool = tc.alloc_tile_pool(name="impb", bufs=1)
    npool = tc.alloc_tile_pool(name="neg", bufs=1)
    scrpool = tc.alloc_tile_pool(name="scr", bufs=2)
    mpool = tc.alloc_tile_pool(name="maskb", bufs=1)
    stpool = tc.alloc_tile_pool(name="stats", bufs=1)
    ps1p = tc.alloc_tile_pool(name="ps1", bufs=2, space="PSUM")
    ps2p = tc.alloc_tile_pool(name="ps2", bufs=3, space="PSUM")
    ps3p = tc.alloc_tile_pool(name="ps3", bufs=3, space="PSUM")

    def cdma(name, shape, dt):
        t = cpool.tile(shape, dt, name=name, tag=name)
        nc.sync.dma_start(out=t[:], in_=d[name][:])
        return t

    c = dict(d=d)
    c["wh1"] = cdma("wh1", [128, 2, MID], F32R)
    c["wl1"] = cdma("wl1", [128, 2, MID], F32R)
    c["t1w"] = cdma("t1w", [128, 9, MID], F32R)
    c["t2w"] = cdma("t2w", [128, 3, MID], F32R)
    c["w2s"] = cdma("w2s", [MID, 3, MID], F32R)
    c["w3p"] = cdma("w3p", [128, C], F32R)
    c["wl3"] = cdma("wl3", [MID, C], F32R)
    c["b1"] = cdma("b1", [MID, 1], F32)
    c["b2"] = cdma("b2", [MID, 1], F32)
    c["b3s"] = cdma("b3s", [128, 2], F32)

    iota_i = cpool.tile([128, 8], I32, name="iotai", tag="iotai")
    nc.gpsimd.iota(iota_i[:], pattern=[[1, 8]], base=0, channel_multiplier=0)
    iotaneg = cpool.tile([128, 8], F32, name="iotan", tag="iotan")
    nc.vector.tensor_copy(iotaneg[:], iota_i[:])
    nc.vector.tensor_scalar(iotaneg[:], iotaneg[:], -1.0, None, op0=OP.mult)
    c["iotaneg"] = iotaneg
    negbig = cpool.tile([128, NBLK], F32, name="negbig", tag="negbig")
    nc.vector.memset(negbig[:], BIGNEG)
    c["negbig"] = negbig
    stgt = cpool.tile([128, 1], F32, name="stgt", tag="stgt")
    nc.vector.memset(stgt[:], -S_TARGET)
    c["stgt"] = stgt
    wh1b = cpool.tile([128, 2, MID], BF16, name="wh1b", tag="wh1b")
    nc.vector.tensor_copy(wh1b[:], c["wh1"][:].bitcast(F32))
    c["wh1b"] = wh1b
    b3s4 = cpool.tile([128, 2], F32, name="b3s4", tag="b3s4")
    nc.vector.tensor_scalar(b3s4[:], c["b3s"][:], 4.0, None, op0=OP.mult)
    c["b3s4"] = b3s4
    tbi = T_BISECT if DBG_TBI < 0 else DBG_TBI
    c["tbi"] = tbi
    ptiles = []
    for t in range(max(tbi, 1)):
        pt_ = cpool.tile([128, 1], F32, name=f"pt{t}", tag=f"ptl{t}")
        nc.vector.memset(pt_[:], -(2.0 ** (-t)))
        ptiles.append(pt_)
    c["ptiles"] = ptiles

    t1t = h1pool.tile([128, HALO * HALO], F32R, name="t1", tag="t1")
    c["t1f"] = t1t[:].rearrange("p (r c) -> p r c", c=HALO)   # [xh1 | xl1]
    t2t = h1pool.tile([128, HALO * HALO], F32R, name="t2", tag="t2")
    c["t2f"] = t2t[:].rearrange("p (r c) -> p r c", c=HALO)   # [xh1 | colshift]
    h1tmp = h1pool.tile([64, HALO * HALO], F32, name="h1tmp", tag="h1tmp")
    c["h1tf"] = h1tmp[:].rearrange("p (r c) -> p r c", c=HALO)
    for tf in (c["t1f"], c["t2f"]):
        for half in (slice(0, 64), slice(64, 128)):
            nc.vector.memset(tf[half, 0:1, :].bitcast(F32), 0.0)
            nc.vector.memset(tf[half, 65:66, :].bitcast(F32), 0.0)
            nc.vector.memset(tf[half, 1:65, 0:1].bitcast(F32), 0.0)
            nc.vector.memset(tf[half, 1:65, 64:66].bitcast(F32), 0.0)

    c["pools"] = [ps3p, ps2p, ps1p, stpool, mpool, scrpool, npool, bpool,
                  ipool, sfpool, h2pool, h1pool, xpool, cpool]
    c.update(xpool=xpool, h2pool=h2pool, sfpool=sfpool, ipool=ipool,
             bpool=bpool, npool=npool, scrpool=scrpool, mpool=mpool,
             stpool=stpool, ps1p=ps1p, ps2p=ps2p, ps3p=ps3p)
    return c


def _build(nc, c, pending):
    d = c["d"]
    t1f, t2f, h1tf = c["t1f"], c["t2f"], c["h1tf"]
    tbi = c["tbi"]

    prev_q = list(pending)

    def drip_prev():
        for _ in range(2):
            if prev_q:
                prev_q.pop(0)()

    state = {0: {}, 1: {}}

    def conv1(img, drip):
        for ci in range(8):
            cs = ci * 512
            xs = []
            for piece, src in (("h", d["fh"]), ("l", d["fl"])):
                for kc in range(2):
                    dt_x = F32R if piece == "h" else BF16
                    xt = c["xpool"].tile([128, 512], dt_x, name="xc", tag=f"x{piece}{kc}")
                    nc.sync.dma_start(
                        out=xt[:], in_=src[img, kc * 128:(kc + 1) * 128, cs:cs + 512])
                    xs.append((piece, kc, xt))
            ps = c["ps1p"].tile([64, 512], F32, name="ps1", tag="ps1")
            mm = []
            for piece, kc, xt in xs:
                if piece == "h":
                    mm.append((c["wh1"][:, kc, :], xt))
                    mm.append((c["wl1"][:, kc, :], xt))
                else:
                    mm.append((c["wh1b"][:, kc, :], xt))
            for i, (lhs, xt) in enumerate(mm):
                nc.tensor.matmul(ps[:], lhs, xt[:], start=(i == 0),
                                 stop=(i == len(mm) - 1))
            psv = ps[:].rearrange("p (r c) -> p r c", c=64)
            r0 = 1 + 8 * ci
            nc.scalar.activation(t1f[0:64, r0:r0 + 8, 1:65], psv, AF.Gelu, bias=c["b1"][:])
            nc.scalar.activation(h1tf[:, r0:r0 + 8, 1:65], psv, AF.Gelu, bias=c["b1"][:])
            # xh1 clones for the T2 (col-shift) pairing on DVE (ACT-bound phase)
            nc.vector.tensor_copy(t2f[0:64, r0:r0 + 8, 1:65],
                                  t1f[0:64, r0:r0 + 8, 1:65].bitcast(F32))
            nc.vector.tensor_copy(t2f[64:128, r0:r0 + 8, 0:64],
                                  t1f[0:64, r0:r0 + 8, 1:65].bitcast(F32))
            nc.vector.tensor_tensor(t1f[64:128, r0:r0 + 8, 1:65],
                                    h1tf[:, r0:r0 + 8, 1:65],
                                    t1f[0:64, r0:r0 + 8, 1:65].bitcast(F32),
                                    op=OP.subtract)
            if ci % 2 == 1:
                drip()

    def conv23(img, drip):
        t4 = c["h2pool"].tile([128, HW], F32R, name="t4", tag="t4")
        h2tmp = c["h2pool"].tile([64, HW], F32, name="h2tmp", tag="h2tmp")
        impp = c["ipool"].tile([128, 2, NBLK], F32, name="impp", tag=f"impp{img}")
        state[img]["impp"] = impp
        for c2q in range(4):
            duo = []
            for dc in range(2):
                c2 = 2 * c2q + dc
                ps = c["ps2p"].tile([64, 512], F32, name="ps2", tag="ps2")
                duo.append((c2, ps))
            for t in range(15):
                for c2, ps in duo:
                    r0 = 8 * c2
                    if t < 9:
                        dy, dx = divmod(t, 3)
                        lhsT = c["t1w"][:, t, :]
                        rhs = t1f[:, r0 + dy:r0 + dy + 8, dx:dx + 64]
                    elif t < 12:
                        dy = t - 9
                        lhsT = c["t2w"][:, dy, :]
                        rhs = t2f[:, r0 + dy:r0 + dy + 8, 0:64]
                    else:
                        dy = t - 12
                        lhsT = c["w2s"][:, dy, :]
                        rhs = t1f[0:64, r0 + dy:r0 + dy + 8, 2:66]
                    nc.tensor.matmul(
                        ps[0:64, :].rearrange("p (r c) -> p r c", c=64),
                        lhsT, rhs, start=(t == 0), stop=(t == 14))
            for c2, ps in duo:
                cs = c2 * 512
                nc.scalar.activation(t4[0:64, cs:cs + 512], ps[:], AF.Gelu, bias=c["b2"][:])
                nc.scalar.activation(h2tmp[:, cs:cs + 512], ps[:], AF.Gelu, bias=c["b2"][:])
                nc.vector.tensor_tensor(t4[64:128, cs:cs + 512],
                                        h2tmp[:, cs:cs + 512],
                                        t4[0:64, cs:cs + 512].bitcast(F32),
                                        op=OP.subtract)
            drip()
            for c2, _ in duo:
                cs = c2 * 512
                sfc = c["sfpool"].tile([128, 2, 512], BF16, name="sfc", tag="sfc")
                for mh in range(2):
                    ps = c["ps3p"].tile([128, 512], F32, name="ps3", tag="ps3")
                    nc.tensor.matmul(ps[:], c["w3p"][:, mh * 128:(mh + 1) * 128],
                                     t4[:, cs:cs + 512], start=True, stop=False)
                    nc.tensor.matmul(ps[:], c["wl3"][:, mh * 128:(mh + 1) * 128],
                                     t4[0:64, cs:cs + 512], start=False, stop=True)
                    nc.scalar.activation(sfc[:, mh, :], ps[:], AF.Identity,
                                         bias=c["b3s"][:, mh:mh + 1])
                    # exact 2x2 block sums straight off psum, + 4*b3 (bias fold)
                    sv = ps[:].rearrange("p (r c w) -> p r c w", c=32, w=2)
                    cpa = c["scrpool"].tile([128, 8, 32], F32, name="cpa", tag="cpa")
                    nc.vector.tensor_copy(cpa[:], sv[:, :, :, 0])
                    cp = c["scrpool"].tile([128, 8, 32], F32, name="cp", tag="cp")
                    nc.vector.tensor_tensor(cp[:], cpa[:], sv[:, :, :, 1],
                                            op=OP.add)
                    cpv = cp[:].rearrange("p (r w) c -> p r w c", w=2)
                    ipc = impp[:, mh, c2 * 128:(c2 + 1) * 128].rearrange(
                        "p (r c) -> p r c", c=32)
                    nc.vector.scalar_tensor_tensor(
                        ipc, cpv[:, :, 0, :], c["b3s4"][:, mh:mh + 1],
                        cpv[:, :, 1, :], op0=OP.add, op1=OP.add)
                nc.sync.dma_start(
                    out=d["scores"][img].rearrange("(t c) w -> c t w", t=2)[:, :, cs:cs + 512],
                    in_=sfc[:])
            drip()

    # ---------------- search (as a list of batch closures) ----------------
    def st(tag, w=2, dt=F32):
        return c["stpool"].tile([128, w], dt, name=tag, tag=tag)

    def search_batches(img):
        if DBG_NOSEARCH:
            return []
        s = state[img]
        batches = []

        def stats_a():
            impp = s["impp"]
            agg = c["stpool"].tile([128, 2, 2], F32, name=f"agg{img}", tag=f"agg{img}")
            for rt in range(2):
                bs6 = c["stpool"].tile([128, 2, 6], F32, name=f"bs{img}{rt}",
                                       tag=f"bs{img}{rt}")
                for g in range(2):
                    nc.vector.bn_stats(bs6[:, g, :], impp[:, rt, g * 512:(g + 1) * 512])
                nc.vector.bn_aggr(agg[:, rt, :], bs6[:])
            for n in ("mu", "sig", "step0", "inv0", "nmsc"):
                s[n] = st(f"{n}{img}")
            nc.vector.tensor_copy(s["mu"][:], agg[:, :, 0])
            nc.scalar.activation(s["sig"][:], agg[:, :, 1], AF.Sqrt)
            nc.vector.tensor_scalar(s["step0"][:], s["sig"][:], (BR_HI - BR_LO) / 4.0,
                                    None, op0=OP.mult)
            nc.vector.reciprocal(s["inv0"][:], s["step0"][:])
            nc.vector.tensor_mul(s["nmsc"][:], s["mu"][:], s["inv0"][:])
            nc.vector.tensor_scalar(s["nmsc"][:], s["nmsc"][:], -1.0, None, op0=OP.mult)

        def stats_b():
            impp = s["impp"]
            impb = c["bpool"].tile([128, 2, NBLK], BF16, name="impb", tag=f"impb{img}")
            for rt in range(2):
                nc.vector.tensor_scalar(impb[:, rt, :], impp[:, rt, :],
                                        s["inv0"][:, rt:rt + 1], s["nmsc"][:, rt:rt + 1],
                                        op0=OP.mult, op1=OP.add)
            negimp = c["npool"].tile([128, 2, NBLK], F32, name="negimp",
                                     tag=f"negimp{img}")
            nc.vector.tensor_scalar(negimp[:], impp[:], -1.0, None, op0=OP.mult)
            s["impb"], s["negimp"] = impb, negimp
            cmid = (BR_LO + BR_HI) / 2.0 / ((BR_HI - BR_LO) / 4.0)
            nm = [st(f"nm{img}{j}", 1) for j in range(2)]
            nc.vector.memset(nm[0][:], -cmid)        # negmid (ACT chain, rt 0)
            s["nm"] = nm
            md = st(f"md{img}", 1)
            nc.vector.memset(md[:], cmid)            # mid (DVE chain, rt 1)
            s["md"] = md

        batches.append(stats_a)
        batches.append(stats_b)

        def iter_batch(t):
            def go():
                # ACT chain (rt 0): count = Sign(v - mid) accumulated
                nm_old, nm_new = s["nm"][t % 2], s["nm"][(t + 1) % 2]
                scr = c["scrpool"].tile([128, NBLK], BF16, name="scr", tag="scrA")
                cntA = st(f"cA{img}", 1)
                sg = st(f"sA{img}", 1)
                nc.scalar.activation(scr[:], s["impb"][:, 0, :], AF.Sign,
                                     bias=nm_old[:], accum_out=cntA[:])
                nc.scalar.activation(sg[:], cntA[:], AF.Sign, bias=c["stgt"][:])
                nc.scalar.activation(nm_new[:], sg[:], AF.Identity, bias=nm_old[:],
                                     scale=c["ptiles"][t][:])
                # DVE chain (rt 1)
                md = s["md"]
                scrD = c["scrpool"].tile([128, NBLK], BF16, name="scr", tag="scrD")
                cntD = st(f"cD{img}", 1)
                dd = st(f"dD{img}", 1, U8)
                ee = st(f"eD{img}", 1)
                nc.vector.tensor_scalar(scrD[:], s["impb"][:, 1, :], md[:], None,
                                        op0=OP.is_le, op1=OP.add, accum_out=cntD[:])
                nc.vector.tensor_scalar(dd[:], cntD[:], float(K_SEARCH), None,
                                        op0=OP.is_ge)
                nc.vector.tensor_scalar(ee[:], dd[:], -(2.0 ** (1 - t)), 2.0 ** (-t),
                                        op0=OP.mult, op1=OP.add)
                nc.vector.tensor_add(md[:], md[:], ee[:])
            return go

        for t in range(tbi):
            batches.append(iter_batch(t))

        def fin_a():
            mid = st(f"mid{img}")
            s["fmid"] = mid
            nm = s["nm"][tbi % 2]
            nc.vector.tensor_scalar(mid[:, 0:1], nm[:], -1.0, None, op0=OP.mult)
            nc.vector.tensor_copy(mid[:, 1:2], s["md"][:])
            for n in ("lofn", "lof", "cntl"):
                s[n] = st(f"{n}{img}")
            nc.vector.tensor_scalar(s["lofn"][:], mid[:],
                                    -(2.0 ** (1 - max(tbi, 1))), None, op0=OP.add)
            nc.vector.tensor_mul(s["lof"][:], s["lofn"][:], s["step0"][:])
            nc.vector.tensor_add(s["lof"][:], s["lof"][:], s["mu"][:])
            s["mles"] = []
            for rt in range(2):
                mle = c["scrpool"].tile([128, NBLK], U8, name="mle", tag=f"mle{rt}")
                s["mles"].append(mle)
                nc.vector.tensor_scalar(mle[:], s["impp"][:, rt, :],
                                        s["lof"][:, rt:rt + 1], None, op0=OP.is_le,
                                        op1=OP.add, accum_out=s["cntl"][:, rt:rt + 1])

        def fin_b():
            for rt in range(2):
                nc.vector.copy_predicated(s["negimp"][:, rt, :], s["mles"][rt][:],
                                          c["negbig"][:])
            s["top8s"] = []
            for rt in range(2):
                top8 = st(f"top8{img}{rt}", 8)
                s["top8s"].append(top8)
                nc.vector.max(out=top8[:], in_=s["negimp"][:, rt, :])

        def fin_c():
            jneg, thrn, thr = (st(f"{n}{img}") for n in ("jneg", "thrn", "thr"))
            s["thr"] = thr
            mneg = st(f"mneg{img}", 2, U8)
            nc.vector.tensor_scalar(jneg[:], s["cntl"][:], -255.0, None, op0=OP.add)
            for rt in range(2):
                eq8 = st(f"eq8{img}{rt}", 8)
                nc.vector.tensor_scalar(eq8[:], c["iotaneg"][:], jneg[:, rt:rt + 1],
                                        None, op0=OP.is_equal)
                nc.vector.tensor_mul(eq8[:], s["top8s"][rt][:], eq8[:])
                nc.vector.tensor_reduce(thrn[:, rt:rt + 1], eq8[:],
                                        axis=mybir.AxisListType.X, op=OP.add)
            nc.vector.tensor_scalar(thr[:], thrn[:], -1.0, None, op0=OP.mult)
            nc.vector.tensor_scalar(mneg[:], s["cntl"][:], 256.0, None, op0=OP.is_ge)
            nc.vector.copy_predicated(thr[:], mneg[:], s["lof"][:])

        def fin_d():
            maskb = c["mpool"].tile([128, 2, NBLK], U8, name="maskb", tag=f"maskb{img}")
            for rt in range(2):
                nc.vector.tensor_scalar(maskb[:, rt, :], s["impp"][:, rt, :],
                                        s["thr"][:, rt:rt + 1], None, op0=OP.is_le)
            nc.sync.dma_start(out=d["mask"][img].rearrange("(t c) w -> c t w", t=2),
                              in_=maskb[:])

        if not DBG_NOFIN:
            batches.extend([fin_a, fin_b, fin_c, fin_d])
        return batches

    # ---------------- schedule ----------------
    conv1(0, drip_prev)
    conv23(0, drip_prev)
    while prev_q:
        prev_q.pop(0)()

    q0 = search_batches(0)

    def drip0():
        for _ in range(2):
            if q0:
                q0.pop(0)()

    conv1(1, drip0)
    conv23(1, drip0)
    while q0:
        q0.pop(0)()

    return search_batches(1)


def _prep_weights(w1, b1, w2, b2, w3, b3):
    w1m = np.ascontiguousarray(w1[:, :, 0, 0].T).astype(np.float32)   # [256, 64]
    wh1, wl1 = _split11(w1m.reshape(2, 128, MID))                     # [2,128,64]
    wh1 = np.ascontiguousarray(wh1.transpose(1, 0, 2))                # [128,2,64]
    wl1 = np.ascontiguousarray(wl1.transpose(1, 0, 2))

    w2t = np.zeros((MID, 9, MID), np.float32)
    for m in range(MID):
        g = m // 8
        for dy in range(3):
            for dx in range(3):
                w2t[g * 8:(g + 1) * 8, 3 * dy + dx, m] = w2[m, :, dy, dx]
    w2h, w2l = _split11(w2t)                                          # [64,9,64]
    t1w = np.concatenate([w2h, w2h], axis=0)                          # [128,9,64]
    t2w = np.stack([np.concatenate([w2l[:, 3 * dy + 0, :],
                                    w2l[:, 3 * dy + 1, :]], axis=0)
                    for dy in range(3)], axis=1)                      # [128,3,64]
    w2s = np.ascontiguousarray(w2l[:, [2, 5, 8], :])                  # [64,3,64]

    w3m = np.ascontiguousarray(w3[:, :, 0, 0].T).astype(np.float32)   # [64, 256]
    wh3, wl3 = _split11(w3m)
    w3p = np.concatenate([wh3, wh3], axis=0)                          # [128,256]

    b3s = np.ascontiguousarray(b3.reshape(2, 128).T).astype(np.float32)
    return dict(wh1=wh1, wl1=wl1, t1w=t1w, t2w=t2w, w2s=w2s,
                w3p=w3p, wl3=wl3,
                b1=b1.reshape(MID, 1).astype(np.float32),
                b2=b2.reshape(MID, 1).astype(np.float32),
                b3s=b3s)


def make_in_maps(inputs):
    wmap = _prep_weights(np.asarray(inputs["w1"]), np.asarray(inputs["b1"]),
                         np.asarray(inputs["w2"]), np.asarray(inputs["b2"]),
                         np.asarray(inputs["w3"]), np.asarray(inputs["b3"]))
    import ml_dtypes
    f = np.asarray(inputs["features"], np.float32).reshape(B, C, HW)
    fh = _round11(f)
    fl = (f - fh).astype(ml_dtypes.bfloat16)
    return [dict(fh=fh[c * IMGS:(c + 1) * IMGS], fl=fl[c * IMGS:(c + 1) * IMGS],
                 **wmap) for c in range(N_CORES)]


_nc_cache = None


def kernel(features, w1, b1, w2, b2, w3, b3, enabled):
    global _nc_cache, LAST_RESULTS
    if not int(np.asarray(enabled)):
        return (np.ones((B, C, H, W), np.float32),
                np.zeros((B, C, H, W), np.float32))
    if _nc_cache is None:
        _nc_cache = build_nc()
    nc = _nc_cache
    in_maps = make_in_maps(dict(features=features, w1=w1, b1=b1, w2=w2, b2=b2,
                                w3=w3, b3=b3))
    res = bass_utils.run_bass_kernel_spmd(nc, in_maps, list(range(N_CORES)),
                                          trace=TRACE)
    LAST_RESULTS = res
    maskb = np.concatenate(
        [np.asarray(res.results[c]["mask"]) for c in range(N_CORES)], 0)
    scores = np.concatenate([res.results[c]["scores"] for c in range(N_CORES)], 0)
    blocks = (maskb != 0).reshape(B, C, 32, 32)
    full = np.broadcast_to(blocks[:, :, :, None, :, None],
                           (B, C, 32, 2, 32, 2)).reshape(B, C, H, W)
    return (full.astype(np.float32),
            scores.reshape(B, C, H, W).astype(np.float32))


if __name__ == "__main__":
    nc = build_nc()
    print("build + compile OK")


# revision 14
# speedup vs baseline: 1.0574x; 1.0574x over previous
"""Trainium2 Bass kernel for nn_EntropyMaskGate (fp32r 3-term split version).

Per core (2 images, batch-sharded over 8 cores):
  conv1 (1x1, 256->64) -> gelu -> grouped 3x3 conv (SAME, 8 groups) -> gelu
  -> conv3 (1x1, 64->256) + bias = entropy_scores -> 2x2 block sums ->
  per-(b,c)-row 256th-smallest threshold -> u8 block mask (host upsamples).

All matmuls run in fp32r (PE rounds operands to 11 explicit mantissa bits,
1 cyc/row vs fp32's 4).  Exactness is recovered with a 3-term split:
    w @ x = wh@xh + wh@xl + wl@xh + O(2^-23)
where wh = rnd11(w), wl = rnd11(w - wh) (host-precomputed, DMA'd straight
into fp32r dram tensors) and xh = rnd11(x), xl = rnd11(x - xh) (features
split on host; h1/h2 split on device: ACT writes gelu to fp32r = xh, DVE
subtracts to an fp32r tile = xl).  Stacking [xh | xl] in the 128-partition
K dim doubles K utilisation for the grouped conv2 (K=64 blocks) and conv3
(K=64), so the 3x cost of the split is mostly absorbed.

Threshold search: per-image 2-chain bisection on normalized bf16 copies;
chain 0 runs self-contained on ACT (Sign count w/ accum + Sign + Identity
mid update), chain 1 self-contained on DVE, so neither blocks the other
engine's queue.  Search work is diced into batches and dripped into later
conv phases: image 0's search hides under image 1's convs, and image 1's
search is handed to the NEXT repeat's image-0 conv phase (steady-state
software pipelining across repeats; the final repeat's tail is flushed at
the end).  An exact f32 top-8 finisher recovers the true threshold.
"""

import numpy as np

import concourse.bass as bass
import concourse.mybir as mybir
from concourse import bacc, bass_utils
from concourse.tile import TileContext

F32 = mybir.dt.float32
F32R = mybir.dt.float32r
BF16 = mybir.dt.bfloat16
I32 = mybir.dt.int32
U8 = mybir.dt.uint8
AF = mybir.ActivationFunctionType
OP = mybir.AluOpType

B, C, H, W = 16, 256, 64, 64
MID, GROUPS = 64, 8
N_CORES = 8
IMGS = B // N_CORES
HW = H * W                   # 4096
NBLK = 1024
KEEP = 256
T_BISECT = 9
BR_LO = -1.05
BR_HI = -0.35
K_SEARCH = 253
S_TARGET = float(NBLK - 2 * K_SEARCH)   # sign-sum equivalent of cnt>=K_SEARCH
BIGNEG = -(2.0 ** 96)
HALO = 66

TRACE = False
LAST_RESULTS = None
import os as _os
DBG_NOSEARCH = int(_os.environ.get("KM_NOSEARCH", "0"))
DBG_TBI = int(_os.environ.get("KM_TBI", "-1"))
DBG_NOFIN = int(_os.environ.get("KM_NOFIN", "0"))


def _round11(a):
    """Round-to-nearest (half-up) keeping 11 explicit mantissa bits."""
    b = np.ascontiguousarray(a, np.float32).view(np.uint32).astype(np.uint64)
    b = ((b + (1 << 11)) >> 12) << 12
    return (b & 0xFFFFFFFF).astype(np.uint32).view(np.float32)


def _split11(a):
    hi = _round11(a)
    lo = _round11(np.asarray(a, np.float32) - hi)
    return hi, lo


def build_nc(repeat=1):
    nc = bacc.Bacc("TRN2", target_bir_lowering=False, debug=False,
                   num_devices=N_CORES)

    d = {}
    d["fh"] = nc.dram_tensor("fh", [IMGS, C, HW], F32R, kind="ExternalInput").ap()
    d["fl"] = nc.dram_tensor("fl", [IMGS, C, HW], BF16, kind="ExternalInput").ap()
    d["wh1"] = nc.dram_tensor("wh1", [128, 2, MID], F32R, kind="ExternalInput").ap()
    d["wl1"] = nc.dram_tensor("wl1", [128, 2, MID], F32R, kind="ExternalInput").ap()
    d["t1w"] = nc.dram_tensor("t1w", [128, 9, MID], F32R, kind="ExternalInput").ap()
    d["t2w"] = nc.dram_tensor("t2w", [128, 3, MID], F32R, kind="ExternalInput").ap()
    d["w2s"] = nc.dram_tensor("w2s", [MID, 3, MID], F32R, kind="ExternalInput").ap()
    d["w3p"] = nc.dram_tensor("w3p", [128, C], F32R, kind="ExternalInput").ap()
    d["wl3"] = nc.dram_tensor("wl3", [MID, C], F32R, kind="ExternalInput").ap()
    d["b1"] = nc.dram_tensor("b1", [MID, 1], F32, kind="ExternalInput").ap()
    d["b2"] = nc.dram_tensor("b2", [MID, 1], F32, kind="ExternalInput").ap()
    d["b3s"] = nc.dram_tensor("b3s", [128, 2], F32, kind="ExternalInput").ap()
    d["scores"] = nc.dram_tensor("scores", [IMGS, C, HW], BF16, kind="ExternalOutput").ap()
    d["mask"] = nc.dram_tensor("mask", [IMGS, C, NBLK], U8, kind="ExternalOutput").ap()

    with TileContext(nc) as tc:
        ctx = _make_ctx(nc, tc, d)
        pending = []
        for _rep in range(repeat):
            pending = _build(nc, ctx, pending)
        for b in pending:
            b()
        for p in ctx["pools"]:
            p.release()
    nc.compile()
    return nc


def _make_ctx(nc, tc, d):
    cpool = tc.alloc_tile_pool(name="consts", bufs=1)
    xpool = tc.alloc_tile_pool(name="x", bufs=2)
    h1pool = tc.alloc_tile_pool(name="h1", bufs=1)
    h2pool = tc.alloc_tile_pool(name="h2", bufs=1)
    sfpool = tc.alloc_tile_pool(name="sf", bufs=2)
    ipool = tc.alloc_tile_pool(name="impp", bufs=1)
    bpool = tc.alloc_tile_pool(name="impb", bufs=1)
    npool = tc.alloc_tile_pool(name="neg", bufs=1)
    scrpool = tc.alloc_tile_pool(name="scr", bufs=2)
    mpool = tc.alloc_tile_pool(name="maskb", bufs=1)
    stpool = tc.alloc_tile_pool(name="stats", bufs=1)
    ps1p = tc.alloc_tile_pool(name="ps1", bufs=2, space="PSUM")
    ps2p = tc.alloc_tile_pool(name="ps2", bufs=3, space="PSUM")
    ps3p = tc.alloc_tile_pool(name="ps3", bufs=3, space="PSUM")

    def cdma(name, shape, dt):
        t = cpool.tile(shape, dt, name=name, tag=name)
        nc.sync.dma_start(out=t[:], in_=d[name][:])
        return t

    c = dict(d=d)
    c["wh1"] = cdma("wh1", [128, 2, MID], F32R)
    c["wl1"] = cdma("wl1", [128, 2, MID], F32R)
    c["t1w"] = cdma("t1w", [128, 9, MID], F32R)
    c["t2w"] = cdma("t2w", [128, 3, MID], F32R)
    c["w2s"] = cdma("w2s", [MID, 3, MID], F32R)
    c["w3p"] = cdma("w3p", [128, C], F32R)
    c["wl3"] = cdma("wl3", [MID, C], F32R)
    c["b1"] = cdma("b1", [MID, 1], F32)
    c["b2"] = cdma("b2", [MID, 1], F32)
    c["b3s"] = cdma("b3s", [128, 2], F32)

    iota_i = cpool.tile([128, 8], I32, name="iotai", tag="iotai")
    nc.gpsimd.iota(iota_i[:], pattern=[[1, 8]], base=0, channel_multiplier=0)
    iotaneg = cpool.tile([128, 8], F32, name="iotan", tag="iotan")
    nc.vector.tensor_copy(iotaneg[:], iota_i[:])
    nc.vector.tensor_scalar(iotaneg[:], iotaneg[:], -1.0, None, op0=OP.mult)
    c["iotaneg"] = iotaneg
    negbig = cpool.tile([128, NBLK], F32, name="negbig", tag="negbig")
    nc.vector.memset(negbig[:], BIGNEG)
    c["negbig"] = negbig
    stgt = cpool.tile([128, 1], F32, name="stgt", tag="stgt")
    nc.vector.memset(stgt[:], -S_TARGET)
    c["stgt"] = stgt
    wh1b = cpool.tile([128, 2, MID], BF16, name="wh1b", tag="wh1b")
    nc.vector.tensor_copy(wh1b[:], c["wh1"][:].bitcast(F32))
    c["wh1b"] = wh1b
    b3s4 = cpool.tile([128, 2], F32, name="b3s4", tag="b3s4")
    nc.vector.tensor_scalar(b3s4[:], c["b3s"][:], 4.0, None, op0=OP.mult)
    c["b3s4"] = b3s4
    tbi = T_BISECT if DBG_TBI < 0 else DBG_TBI
    c["tbi"] = tbi
    ptiles = []
    for t in range(max(tbi, 1)):
        pt_ = cpool.tile([128, 1], F32, name=f"pt{t}", tag=f"ptl{t}")
        nc.vector.memset(pt_[:], -(2.0 ** (-t)))
        ptiles.append(pt_)
    c["ptiles"] = ptiles

    t1t = h1pool.tile([128, HALO * HALO], F32R, name="t1", tag="t1")
    c["t1f"] = t1t[:].rearrange("p (r c) -> p r c", c=HALO)   # [xh1 | xl1]
    t2t = h1pool.tile([128, HALO * HALO], F32R, name="t2", tag="t2")
    c["t2f"] = t2t[:].rearrange("p (r c) -> p r c", c=HALO)   # [xh1 | colshift]
    h1tmp = h1pool.tile([64, HALO * HALO], F32, name="h1tmp", tag="h1tmp")
    c["h1tf"] = h1tmp[:].rearrange("p (r c) -> p r c", c=HALO)
    for tf in (c["t1f"], c["t2f"]):
        for half in (slice(0, 64), slice(64, 128)):
            nc.vector.memset(tf[half, 0:1, :].bitcast(F32), 0.0)
            nc.vector.memset(tf[half, 65:66, :].bitcast(F32), 0.0)
            nc.vector.memset(tf[half, 1:65, 0:1].bitcast(F32), 0.0)
            nc.vector.memset(tf[half, 1:65, 64:66].bitcast(F32), 0.0)

    c["pools"] = [ps3p, ps2p, ps1p, stpool, mpool, scrpool, npool, bpool,
                  ipool, sfpool, h2pool, h1pool, xpool, cpool]
    c.update(xpool=xpool, h2pool=h2pool, sfpool=sfpool, ipool=ipool,
             bpool=bpool, npool=npool, scrpool=scrpool, mpool=mpool,
             stpool=stpool, ps1p=ps1p, ps2p=ps2p, ps3p=ps3p)
    return c


def _build(nc, c, pending):
    d = c["d"]
    t1f, t2f, h1tf = c["t1f"], c["t2f"], c["h1tf"]
    tbi = c["tbi"]

    prev_q = list(pending)

    def drip_prev():
        if prev_q:
            prev_q.pop(0)()

    state = {0: {}, 1: {}}

    def conv1(img, drip):
        for ci in range(8):
            cs = ci * 512
            xs = []
            for piece, src in (("h", d["fh"]), ("l", d["fl"])):
                for kc in range(2):
                    dt_x = F32R if piece == "h" else BF16
                    xt = c["xpool"].tile([128, 512], dt_x, name="xc", tag=f"x{piece}{kc}")
                    nc.sync.dma_start(
                        out=xt[:], in_=src[img, kc * 128:(kc + 1) * 128, cs:cs + 512])
                    xs.append((piece, kc, xt))
            ps = c["ps1p"].tile([64, 512], F32, name="ps1", tag="ps1")
            mm = []
            for piece, kc, xt in xs:
                if piece == "h":
                    mm.append((c["wh1"][:, kc, :], xt))
                    mm.append((c["wl1"][:, kc, :], xt))
                else:
                    mm.append((c["wh1b"][:, kc, :], xt))
            for i, (lhs, xt) in enumerate(mm):
                nc.tensor.matmul(ps[:], lhs, xt[:], start=(i == 0),
                                 stop=(i == len(mm) - 1))
            psv = ps[:].rearrange("p (r c) -> p r c", c=64)
            r0 = 1 + 8 * ci
            nc.scalar.activation(t1f[0:64, r0:r0 + 8, 1:65], psv, AF.Gelu, bias=c["b1"][:])
            nc.scalar.activation(h1tf[:, r0:r0 + 8, 1:65], psv, AF.Gelu, bias=c["b1"][:])
            # xh1 clones for the T2 (col-shift) pairing on DVE (ACT-bound phase)
            nc.vector.tensor_copy(t2f[0:64, r0:r0 + 8, 1:65],
                                  t1f[0:64, r0:r0 + 8, 1:65].bitcast(F32))
            nc.vector.tensor_copy(t2f[64:128, r0:r0 + 8, 0:64],
                                  t1f[0:64, r0:r0 + 8, 1:65].bitcast(F32))
            nc.vector.tensor_tensor(t1f[64:128, r0:r0 + 8, 1:65],
                                    h1tf[:, r0:r0 + 8, 1:65],
                                    t1f[0:64, r0:r0 + 8, 1:65].bitcast(F32),
                                    op=OP.subtract)
            if ci % 2 == 1:
                drip()

    def conv23(img, drip):
        t4 = c["h2pool"].tile([128, HW], F32R, name="t4", tag="t4")
        h2tmp = c["h2pool"].tile([64, HW], F32, name="h2tmp", tag="h2tmp")
        impp = c["ipool"].tile([128, 2, NBLK], F32, name="impp", tag=f"impp{img}")
        state[img]["impp"] = impp
        for c2q in range(4):
            duo = []
            for dc in range(2):
                c2 = 2 * c2q + dc
                ps = c["ps2p"].tile([64, 512], F32, name="ps2", tag="ps2")
                duo.append((c2, ps))
            for t in range(15):
                for c2, ps in duo:
                    r0 = 8 * c2
                    if t < 9:
                        dy, dx = divmod(t, 3)
                        lhsT = c["t1w"][:, t, :]
                        rhs = t1f[:, r0 + dy:r0 + dy + 8, dx:dx + 64]
                    elif t < 12:
                        dy = t - 9
                        lhsT = c["t2w"][:, dy, :]
                        rhs = t2f[:, r0 + dy:r0 + dy + 8, 0:64]
                    else:
                        dy = t - 12
                        lhsT = c["w2s"][:, dy, :]
                        rhs = t1f[0:64, r0 + dy:r0 + dy + 8, 2:66]
                    nc.tensor.matmul(
                        ps[0:64, :].rearrange("p (r c) -> p r c", c=64),
                        lhsT, rhs, start=(t == 0), stop=(t == 14))
            for c2, ps in duo:
                cs = c2 * 512
                nc.scalar.activation(t4[0:64, cs:cs + 512], ps[:], AF.Gelu, bias=c["b2"][:])
                nc.scalar.activation(h2tmp[:, cs:cs + 512], ps[:], AF.Gelu, bias=c["b2"][:])
                nc.vector.tensor_tensor(t4[64:128, cs:cs + 512],
                                        h2tmp[:, cs:cs + 512],
                                        t4[0:64, cs:cs + 512].bitcast(F32),
                                        op=OP.subtract)
            drip()
            for c2, _ in duo:
                cs = c2 * 512
                sfc = c["sfpool"].tile([128, 2, 512], BF16, name="sfc", tag="sfc")
                for mh in range(2):
                    ps = c["ps3p"].tile([128, 512], F32, name="ps3", tag="ps3")
                    nc.tensor.matmul(ps[:], c["w3p"][:, mh * 128:(mh + 1) * 128],
                                     t4[:, cs:cs + 512], start=True, stop=False)
                    nc.tensor.matmul(ps[:], c["wl3"][:, mh * 128:(mh + 1) * 128],
                                     t4[0:64, cs:cs + 512], start=False, stop=True)
                    nc.scalar.activation(sfc[:, mh, :], ps[:], AF.Identity,
                                         bias=c["b3s"][:, mh:mh + 1])
                    # exact 2x2 block sums straight off psum, + 4*b3 (bias fold)
                    sv = ps[:].rearrange("p (r c w) -> p r c w", c=32, w=2)
                    cpa = c["scrpool"].tile([128, 8, 32], F32, name="cpa", tag="cpa")
                    nc.vector.tensor_copy(cpa[:], sv[:, :, :, 0])
                    cp = c["scrpool"].tile([128, 8, 32], F32, name="cp", tag="cp")
                    nc.vector.tensor_tensor(cp[:], cpa[:], sv[:, :, :, 1],
                                            op=OP.add)
                    cpv = cp[:].rearrange("p (r w) c -> p r w c", w=2)
                    ipc = impp[:, mh, c2 * 128:(c2 + 1) * 128].rearrange(
                        "p (r c) -> p r c", c=32)
                    nc.vector.scalar_tensor_tensor(
                        ipc, cpv[:, :, 0, :], c["b3s4"][:, mh:mh + 1],
                        cpv[:, :, 1, :], op0=OP.add, op1=OP.add)
                nc.sync.dma_start(
                    out=d["scores"][img].rearrange("(t c) w -> c t w", t=2)[:, :, cs:cs + 512],
                    in_=sfc[:])
            drip()

    # ---------------- search (as a list of batch closures) ----------------
    def st(tag, w=2, dt=F32):
        return c["stpool"].tile([128, w], dt, name=tag, tag=tag)

    def search_batches(img):
        if DBG_NOSEARCH:
            return []
        s = state[img]
        batches = []

        def stats_a():
            impp = s["impp"]
            agg = c["stpool"].tile([128, 2, 2], F32, name=f"agg{img}", tag=f"agg{img}")
            for rt in range(2):
                bs6 = c["stpool"].tile([128, 2, 6], F32, name=f"bs{img}{rt}",
                                       tag=f"bs{img}{rt}")
                for g in range(2):
                    nc.vector.bn_stats(bs6[:, g, :], impp[:, rt, g * 512:(g + 1) * 512])
                nc.vector.bn_aggr(agg[:, rt, :], bs6[:])
            for n in ("mu", "sig", "step0", "inv0", "nmsc"):
                s[n] = st(f"{n}{img}")
            nc.vector.tensor_copy(s["mu"][:], agg[:, :, 0])
            nc.scalar.activation(s["sig"][:], agg[:, :, 1], AF.Sqrt)
            nc.vector.tensor_scalar(s["step0"][:], s["sig"][:], (BR_HI - BR_LO) / 4.0,
                                    None, op0=OP.mult)
            nc.vector.reciprocal(s["inv0"][:], s["step0"][:])
            nc.vector.tensor_mul(s["nmsc"][:], s["mu"][:], s["inv0"][:])
            nc.vector.tensor_scalar(s["nmsc"][:], s["nmsc"][:], -1.0, None, op0=OP.mult)

        def stats_b():
            impp = s["impp"]
            impb = c["bpool"].tile([128, 2, NBLK], BF16, name="impb", tag=f"impb{img}")
            for rt in range(2):
                nc.vector.tensor_scalar(impb[:, rt, :], impp[:, rt, :],
                                        s["inv0"][:, rt:rt + 1], s["nmsc"][:, rt:rt + 1],
                                        op0=OP.mult, op1=OP.add)
            negimp = c["npool"].tile([128, 2, NBLK], F32, name="negimp",
                                     tag=f"negimp{img}")
            nc.vector.tensor_scalar(negimp[:], impp[:], -1.0, None, op0=OP.mult)
            s["impb"], s["negimp"] = impb, negimp
            cmid = (BR_LO + BR_HI) / 2.0 / ((BR_HI - BR_LO) / 4.0)
            nm = [st(f"nm{img}{j}", 1) for j in range(2)]
            nc.vector.memset(nm[0][:], -cmid)        # negmid (ACT chain, rt 0)
            s["nm"] = nm
            md = st(f"md{img}", 1)
            nc.vector.memset(md[:], cmid)            # mid (DVE chain, rt 1)
            s["md"] = md

        batches.append(stats_a)
        batches.append(stats_b)

        def iter_batch(t):
            def go():
                # ACT chain (rt 0): count = Sign(v - mid) accumulated
                nm_old, nm_new = s["nm"][t % 2], s["nm"][(t + 1) % 2]
                scr = c["scrpool"].tile([128, NBLK], BF16, name="scr", tag="scrA")
                cntA = st(f"cA{img}", 1)
                sg = st(f"sA{img}", 1)
                nc.scalar.activation(scr[:], s["impb"][:, 0, :], AF.Sign,
                                     bias=nm_old[:], accum_out=cntA[:])
                nc.scalar.activation(sg[:], cntA[:], AF.Sign, bias=c["stgt"][:])
                nc.scalar.activation(nm_new[:], sg[:], AF.Identity, bias=nm_old[:],
                                     scale=c["ptiles"][t][:])
                # DVE chain (rt 1)
                md = s["md"]
                scrD = c["scrpool"].tile([128, NBLK], BF16, name="scr", tag="scrD")
                cntD = st(f"cD{img}", 1)
                dd = st(f"dD{img}", 1, U8)
                ee = st(f"eD{img}", 1)
                nc.vector.tensor_scalar(scrD[:], s["impb"][:, 1, :], md[:], None,
                                        op0=OP.is_le, op1=OP.add, accum_out=cntD[:])
                nc.vector.tensor_scalar(dd[:], cntD[:], float(K_SEARCH), None,
                                        op0=OP.is_ge)
                nc.vector.tensor_scalar(ee[:], dd[:], -(2.0 ** (1 - t)), 2.0 ** (-t),
                                        op0=OP.mult, op1=OP.add)
                nc.vector.tensor_add(md[:], md[:], ee[:])
            return go

        for t in range(tbi):
            batches.append(iter_batch(t))

        def fin_a():
            mid = st(f"mid{img}")
            s["fmid"] = mid
            nm = s["nm"][tbi % 2]
            nc.vector.tensor_scalar(mid[:, 0:1], nm[:], -1.0, None, op0=OP.mult)
            nc.vector.tensor_copy(mid[:, 1:2], s["md"][:])
            for n in ("lofn", "lof", "cntl"):
                s[n] = st(f"{n}{img}")
            nc.vector.tensor_scalar(s["lofn"][:], mid[:],
                                    -(2.0 ** (1 - max(tbi, 1))), None, op0=OP.add)
            nc.vector.tensor_mul(s["lof"][:], s["lofn"][:], s["step0"][:])
            nc.vector.tensor_add(s["lof"][:], s["lof"][:], s["mu"][:])
            s["mles"] = []
            for rt in range(2):
                mle = c["scrpool"].tile([128, NBLK], U8, name="mle", tag=f"mle{rt}")
                s["mles"].append(mle)
                nc.vector.tensor_scalar(mle[:], s["impp"][:, rt, :],
                                        s["lof"][:, rt:rt + 1], None, op0=OP.is_le,
                                        op1=OP.add, accum_out=s["cntl"][:, rt:rt + 1])

        def fin_b():
            for rt in range(2):
                nc.vector.copy_predicated(s["negimp"][:, rt, :], s["mles"][rt][:],
                                          c["negbig"][:])
            s["top8s"] = []
            for rt in range(2):
                top8 = st(f"top8{img}{rt}", 8)
                s["top8s"].append(top8)
                nc.vector.max(out=top8[:], in_=s["negimp"][:, rt, :])

        def fin_c():
            jneg, thrn, thr = (st(f"{n}{img}") for n in ("jneg", "thrn", "thr"))
            s["thr"] = thr
            mneg = st(f"mneg{img}", 2, U8)
            nc.vector.tensor_scalar(jneg[:], s["cntl"][:], -255.0, None, op0=OP.add)
            for rt in range(2):
                eq8 = st(f"eq8{img}{rt}", 8)
                nc.vector.tensor_scalar(eq8[:], c["iotaneg"][:], jneg[:, rt:rt + 1],
                                        None, op0=OP.is_equal)
                nc.vector.tensor_mul(eq8[:], s["top8s"][rt][:], eq8[:])
                nc.vector.tensor_reduce(thrn[:, rt:rt + 1], eq8[:],
                                        axis=mybir.AxisListType.X, op=OP.add)
            nc.vector.tensor_scalar(thr[:], thrn[:], -1.0, None, op0=OP.mult)
            nc.vector.tensor_scalar(mneg[:], s["cntl"][:], 256.0, None, op0=OP.is_ge)
            nc.vector.copy_predicated(thr[:], mneg[:], s["lof"][:])

        def fin_d():
            maskb = c["mpool"].tile([128, 2, NBLK], U8, name="maskb", tag=f"maskb{img}")
            for rt in range(2):
                nc.vector.tensor_scalar(maskb[:, rt, :], s["impp"][:, rt, :],
                                        s["thr"][:, rt:rt + 1], None, op0=OP.is_le)
            nc.sync.dma_start(out=d["mask"][img].rearrange("(t c) w -> c t w", t=2),
                              in_=maskb[:])

        if not DBG_NOFIN:
            batches.extend([fin_a, fin_b, fin_c, fin_d])
        return batches

    # ---------------- schedule ----------------
    conv1(0, drip_prev)
    conv23(0, drip_prev)
    while prev_q:
        prev_q.pop(0)()

    q0 = search_batches(0)

    def drip0():
        if q0:
            q0.pop(0)()

    conv1(1, drip0)
    conv23(1, drip0)
    while q0:
        q0.pop(0)()

    return search_batches(1)


def _prep_weights(w1, b1, w2, b2, w3, b3):
    w1m = np.ascontiguousarray(w1[:, :, 0, 0].T).astype(np.float32)   # [256, 64]
    wh1, wl1 = _split11(w1m.reshape(2, 128, MID))                     # [2,128,64]
    wh1 = np.ascontiguousarray(wh1.transpose(1, 0, 2))                # [128,2,64]
    wl1 = np.ascontiguousarray(wl1.transpose(1, 0, 2))

    w2t = np.zeros((MID, 9, MID), np.float32)
    for m in range(MID):
        g = m // 8
        for dy in range(3):
            for dx in range(3):
                w2t[g * 8:(g + 1) * 8, 3 * dy + dx, m] = w2[m, :, dy, dx]
    w2h, w2l = _split11(w2t)                                          # [64,9,64]
    t1w = np.concatenate([w2h, w2h], axis=0)                          # [128,9,64]
    t2w = np.stack([np.concatenate([w2l[:, 3 * dy + 0, :],
                                    w2l[:, 3 * dy + 1, :]], axis=0)
                    for dy in range(3)], axis=1)                      # [128,3,64]
    w2s = np.ascontiguousarray(w2l[:, [2, 5, 8], :])                  # [64,3,64]

    w3m = np.ascontiguousarray(w3[:, :, 0, 0].T).astype(np.float32)   # [64, 256]
    wh3, wl3 = _split11(w3m)
    w3p = np.concatenate([wh3, wh3], axis=0)                          # [128,256]

    b3s = np.ascontiguousarray(b3.reshape(2, 128).T).astype(np.float32)
    return dict(wh1=wh1, wl1=wl1, t1w=t1w, t2w=t2w, w2s=w2s,
                w3p=w3p, wl3=wl3,
                b1=b1.reshape(MID, 1).astype(np.float32),
                b2=b2.reshape(MID, 1).astype(np.float32),
                b3s=b3s)


def make_in_maps(inputs):
    wmap = _prep_weights(np.asarray(inputs["w1"]), np.asarray(inputs["b1"]),
                         np.asarray(inputs["w2"]), np.asarray(inputs["b2"]),
                         np.asarray(inputs["w3"]), np.asarray(inputs["b3"]))
    import ml_dtypes
    f = np.asarray(inputs["features"], np.float32).reshape(B, C, HW)
    fh = _round11(f)
    fl = (f - fh).astype(ml_dtypes.bfloat16)
    return [dict(fh=fh[c * IMGS:(c + 1) * IMGS], fl=fl[c * IMGS:(c + 1) * IMGS],
                 **wmap) for c in range(N_CORES)]


_nc_cache = None


def kernel(features, w1, b1, w2, b2, w3, b3, enabled):
    global _nc_cache, LAST_RESULTS
    if not int(np.asarray(enabled)):
        return (np.ones((B, C, H, W), np.float32),
                np.zeros((B, C, H, W), np.float32))
    if _nc_cache is None:
        _nc_cache = build_nc()
    nc = _nc_cache
    in_maps = make_in_maps(dict(features=features, w1=w1, b1=b1, w2=w2, b2=b2,
                                w3=w3, b3=b3))
    res = bass_utils.run_bass_kernel_spmd(nc, in_maps, list(range(N_CORES)),
                                          trace=TRACE)
    LAST_RESULTS = res
    maskb = np.concatenate(
        [np.asarray(res.results[c]["mask"]) for c in range(N_CORES)], 0)
    scores = np.concatenate([res.results[c]["scores"] for c in range(N_CORES)], 0)
    blocks = (maskb != 0).reshape(B, C, 32, 32)
    full = np.broadcast_to(blocks[:, :, :, None, :, None],
                           (B, C, 32, 2, 32, 2)).reshape(B, C, H, W)
    return (full.astype(np.float32),
            scores.reshape(B, C, H, W).astype(np.float32))


if __name__ == "__main__":
    nc = build_nc()
    print("build + compile OK")
